# revision 41
# baseline (speedup 1.0000x reference)
# Linear-attention layer (phi = elu+1) on 8 Trainium2 NeuronCores.
#
# Reference computation (per batch b):
#   q = x @ Wq + bq ; k = x @ Wk + bk ; v = x @ Wv + bv      [S, DM] each
#   kv[h] = phi(k_h)^T @ v_h          (sum over ALL of S)    [HD, HD]
#   attn_h = phi(q_h) @ kv[h]                                [S, HD]
#   out = attn @ Wo + bo                                     [S, DM]
#
# Sharding: sequence-parallel. Core c owns S/8 = 512 positions of every
# batch (2048 rows total). kv is a sum over sequence -> each core computes
# a partial kv over its rows, four per-batch 128 KiB AllReduces combine
# them (fired as each batch finishes, hiding rendezvous skew), and every
# core finishes its own rows through attn + out_proj. Output rows are
# disjoint across cores, so no other communication is needed.
#
# phi(t) = elu(t) + 1 = exp(min(t, 0)) + relu(t)
#
# Numerics: x/W cast to bf16 on host, matmuls accumulate in fp32 PSUM,
# kv state reduced in bf16 (validated ~3.5e-3 scaled absmax vs fp32 ref).

import numpy as np
import ml_dtypes

B, S, DM, H = 4, 4096, 1024, 16
HD = DM // H          # 64
N_CORES = 8
P = 128
SC = S // N_CORES     # 512 sequence positions per core
R = B * SC            # 2048 rows per core
KC = DM // P          # 8 contraction chunks
NB = SC               # rows per batch on a core (512)
SCB = NB // P         # s-chunks per batch (4)
SCH = R // P          # s-chunks total (16)
NT_R = R // 512       # 512-wide row tiles (4)
ND = DM // 512        # 512-wide feature tiles (2)

_cache = {}


def _build(has_bias):
    import concourse.mybir as mybir
    import concourse.tile as tile
    from concourse import bacc

    fp32 = mybir.dt.float32
    bf16 = mybir.dt.bfloat16
    AF = mybir.ActivationFunctionType
    ALU = mybir.AluOpType

    nc = bacc.Bacc("TRN2", target_bir_lowering=False, debug=False,
                   num_devices=N_CORES)

    x_d = nc.dram_tensor("x", [DM, R], bf16, kind="ExternalInput").ap()
    wq_d = nc.dram_tensor("wq", [DM, DM], bf16, kind="ExternalInput").ap()
    wk_d = nc.dram_tensor("wk", [DM, DM], bf16, kind="ExternalInput").ap()
    wv_d = nc.dram_tensor("wv", [DM, DM], bf16, kind="ExternalInput").ap()
    wo_d = nc.dram_tensor("wo", [DM, DM], bf16, kind="ExternalInput").ap()
    bqc_d = nc.dram_tensor("bqc", [P, KC], fp32, kind="ExternalInput").ap()
    if has_bias:
        bk_d = nc.dram_tensor("bk2", [1, DM], bf16, kind="ExternalInput").ap()
        bv_d = nc.dram_tensor("bv2", [1, DM], bf16, kind="ExternalInput").ap()
        bo_d = nc.dram_tensor("bo2", [1, DM], bf16, kind="ExternalInput").ap()
    out_d = nc.dram_tensor("out", [R, DM], fp32, kind="ExternalOutput").ap()
    with tile.TileContext(nc) as tc:
        with (
            tc.tile_pool(name="big", bufs=1) as big,
            tc.tile_pool(name="stream", bufs=4) as stream,
            tc.tile_pool(name="tmp", bufs=2) as tmpp,
            tc.tile_pool(name="outp", bufs=3) as outp,
            tc.tile_pool(name="psum", bufs=5, space="PSUM") as psum,
            tc.tile_pool(name="kvps", bufs=2, space="PSUM") as kvps,
            tc.tile_pool(name="dram", bufs=2, space="DRAM") as dram,
        ):
            # ---------------- persistent tiles ----------------
            # wk/wv live only through phase 1 (own pool, closed after);
            # attnt is allocated afterwards and reuses their space.
            wkv_pool = tc.tile_pool(name="wkv", bufs=1)
            wkv = wkv_pool.__enter__()
            xt = [big.tile([P, KC, 512], bf16, tag=f"xt{rg}", name=f"xt{rg}")
                  for rg in range(NT_R)]          # x^T, split by row group
            # weights split into 512-wide halves so consumers unblock after
            # 1 MiB instead of 2 MiB of DMA
            wq = [big.tile([P, KC, 512], bf16, tag=f"wq{h}", name=f"wq{h}")
                  for h in range(2)]
            wk = [wkv.tile([P, KC, 512], bf16, tag=f"wk{h}", name=f"wk{h}")
                  for h in range(2)]
            wv = [wkv.tile([P, KC, 512], bf16, tag=f"wv{h}", name=f"wv{h}")
                  for h in range(2)]
            wo = [big.tile([P, KC, 512], bf16, tag=f"wo{h}", name=f"wo{h}")
                  for h in range(2)]
            phiq = big.tile([P, KC, R], bf16, tag="phiq")    # phi(q)^T
            # kv state: head-pair stacked on partitions (even head rows 0:64,
            # odd head rows 64:128); column slot (b*8 + pair)*64
            kv_sb = [big.tile([P, (H // 2) * HD], bf16, tag=f"kv{b}",
                              name=f"kv{b}") for b in range(B)]
            kv_rd = [big.tile([P, (H // 2) * HD], bf16, tag=f"kvr{b}",
                              name=f"kvr{b}") for b in range(B)]
            # block-diag expansion of kv_rd: per (b, pair) a [128,128] block
            # with kv_even at (0:64, 0:64), kv_odd at (64:128, 64:128)
            kv_bd = [big.tile([P, (H // 2) * P], bf16, tag=f"kvbd{b}",
                              name=f"kvbd{b}") for b in range(B)]
            bqc = big.tile([P, KC], fp32, tag="bqc")
            if has_bias:
                bk2 = big.tile([1, DM], bf16, tag="bk2")
                bv2 = big.tile([1, DM], bf16, tag="bv2")
                bo2 = big.tile([1, DM], bf16, tag="bo2")
            ones = big.tile([1, P], bf16, tag="ones")
            zrow = big.tile([1, 512], bf16, tag="zrow")

            KVB = (H // 2) * HD  # 512 columns of kv state per batch
            kv_in = [dram.tile([P, KVB], bf16, tag=f"kvi{b}", name=f"kvi{b}")
                     for b in range(B)]
            kv_out = [dram.tile([P, KVB], bf16, tag=f"kvo{b}", name=f"kvo{b}")
                      for b in range(B)]

            def s512(n):
                return slice(n * 512, (n + 1) * 512)

            # ---------------- loads ----------------
            # x arrives pre-transposed from the host: plain contiguous
            # loads, row-group split so phase 1 unblocks early. Everything
            # rides the sync HWDGE queue in consumption order (keeping the
            # scalar queue free: HWDGE descriptor generation on the scalar
            # sequencer starves ACT compute dispatch).
            xt_dr = x_d.rearrange("(c p) r -> p c r", p=P)
            nc.sync.dma_start(xt[0][:], xt_dr[:, :, s512(0)])
            for h in range(2):
                for w_sb, w_dr in ((wk, wk_d), (wv, wv_d)):
                    for c in range(KC):
                        nc.sync.dma_start(
                            w_sb[h][:, c, :],
                            w_dr[c * P:(c + 1) * P, s512(h)])
            for rg in range(1, NT_R):
                nc.sync.dma_start(xt[rg][:], xt_dr[:, :, s512(rg)])
            for w_sb, w_dr in ((wq, wq_d), (wo, wo_d)):
                for h in range(2):
                    for c in range(KC):
                        nc.sync.dma_start(
                            w_sb[h][:, c, :],
                            w_dr[c * P:(c + 1) * P, s512(h)])
            nc.gpsimd.dma_start(bqc[:], bqc_d)
            if has_bias:
                nc.gpsimd.dma_start(bk2[:], bk_d)
                nc.gpsimd.dma_start(bv2[:], bv_d)
                nc.gpsimd.dma_start(bo2[:], bo_d)
            nc.gpsimd.memset(ones[:], 1.0)
            nc.gpsimd.memset(zrow[:], 0.0)
            for b in range(B):
                nc.gpsimd.memset(kv_bd[b][:], 0.0)

            # ---------- phase 1: k/v projections + phi(k) + partial kv ----------
            for b in range(B):
                kvp = [kvps.tile([P, 512], fp32, tag="kvp0", name="kvp0",
                                 bufs=2),
                       kvps.tile([P, 512], fp32, tag="kvp1", name="kvp1",
                                 bufs=1)]
                for j in (0, 1):
                    # start=True zeroes the whole PSUM bank; do it exactly once
                    # per bank (full-width) so the per-head slot matmuls below
                    # can all accumulate with start=False.
                    nc.tensor.matmul(kvp[j][:], lhsT=ones[:], rhs=zrow[:],
                                     start=True, stop=False)
                for sc in range(SCB):
                    g = b * SCB + sc
                    kch = stream.tile([P, DM], bf16, tag="kch")
                    vch = stream.tile([P, DM], bf16, tag="vch")
                    for n in range(ND):
                        kps = psum.tile([P, 512], fp32, tag="pp")
                        vps = psum.tile([P, 512], fp32, tag="pp")
                        for kc in range(KC):
                            nc.tensor.matmul(
                                kps[:],
                                lhsT=xt[g // 4][:, kc,
                                                (g % 4) * P:(g % 4 + 1) * P],
                                rhs=wk[n][:, kc, :],
                                start=(kc == 0),
                                stop=(not has_bias and kc == KC - 1))
                        if has_bias:
                            nc.tensor.matmul(kps[:], lhsT=ones[:],
                                             rhs=bk2[:, s512(n)],
                                             start=False, stop=True)
                        for kc in range(KC):
                            nc.tensor.matmul(
                                vps[:],
                                lhsT=xt[g // 4][:, kc,
                                                (g % 4) * P:(g % 4 + 1) * P],
                                rhs=wv[n][:, kc, :],
                                start=(kc == 0),
                                stop=(not has_bias and kc == KC - 1))
                        if has_bias:
                            nc.tensor.matmul(vps[:], lhsT=ones[:],
                                             rhs=bv2[:, s512(n)],
                                             start=False, stop=True)
                        # phi(k) = exp(min(k,0)) + relu(k)
                        ut = tmpp.tile([P, 512], bf16, tag="u")
                        nc.vector.tensor_scalar_min(out=ut[:], in0=kps[:],
                                                    scalar1=0.0)
                        rt = tmpp.tile([P, 512], bf16, tag="r")
                        nc.scalar.activation(out=rt[:], in_=kps[:], func=AF.Relu)
                        nc.vector.tensor_copy(out=vch[:, s512(n)], in_=vps[:])
                        et = tmpp.tile([P, 512], bf16, tag="e")
                        nc.scalar.activation(out=et[:], in_=ut[:], func=AF.Exp)
                        nc.vector.tensor_add(out=kch[:, s512(n)], in0=et[:],
                                             in1=rt[:])
                    for pr in range(H // 2):
                        j, col = pr // 4, (pr % 4) * P
                        # full pair x pair cross-product; diagonal 64x64
                        # blocks are the two heads' kv states
                        nc.tensor.matmul(
                            kvp[j][:, col:col + P],
                            lhsT=kch[:, pr * P:(pr + 1) * P],
                            rhs=vch[:, pr * P:(pr + 1) * P],
                            start=False,
                            stop=(sc == SCB - 1 and pr % 4 == 3),
                            skip_group_check=True)
                for h in range(H):
                    pr = h // 2
                    j, col = pr // 4, (pr % 4) * P + (h % 2) * HD
                    rows = slice((h % 2) * HD, (h % 2 + 1) * HD)
                    slot = pr * HD
                    nc.vector.tensor_copy(
                        out=kv_sb[b][rows, slot:slot + HD],
                        in_=kvp[j][rows, col:col + HD])
            # wk/wv dead from here; free their SBUF for attnt
            wkv_pool.__exit__(None, None, None)
            attnt = big.tile([P, KC, R], bf16, tag="attnt")  # attn^T


            # ---------- phase 3: q^T projection + phi ----------
            for m in range(KC):
                for nt in range(NT_R):
                    qps = psum.tile([P, 512], fp32, tag="pp")
                    for kc in range(KC):
                        nc.tensor.matmul(
                            qps[:],
                            lhsT=wq[m // 4][:, kc, (m % 4) * P:(m % 4 + 1) * P],
                            rhs=xt[nt][:, kc, :],
                            start=(kc == 0), stop=(kc == KC - 1))
                    ut = tmpp.tile([P, 512], bf16, tag="u")
                    nc.vector.tensor_scalar(out=ut[:], in0=qps[:],
                                            scalar1=bqc[:, m:m + 1],
                                            scalar2=0.0,
                                            op0=ALU.add, op1=ALU.min)
                    rt = tmpp.tile([P, 512], bf16, tag="r")
                    nc.scalar.activation(out=rt[:], in_=qps[:], func=AF.Relu,
                                         bias=bqc[:, m:m + 1], scale=1.0)
                    et = tmpp.tile([P, 512], bf16, tag="e")
                    nc.scalar.activation(out=et[:], in_=ut[:], func=AF.Exp)
                    nc.vector.tensor_add(out=phiq[:, m, s512(nt)], in0=et[:],
                                         in1=rt[:])

            # per-batch AllReduces: each fires as soon as that batch's
            # partial kv is ready (data deps gate execution, not emission
            # order), hiding rendezvous skew under remaining compute
            for b in range(B):
                nc.gpsimd.dma_start(kv_in[b][:], kv_sb[b][:])
                nc.gpsimd.collective_compute(
                    "AllReduce",
                    mybir.AluOpType.add,
                    replica_groups=[list(range(N_CORES))],
                    ins=[kv_in[b].opt()],
                    outs=[kv_out[b].opt()],
                )
                nc.gpsimd.dma_start(kv_rd[b][:], kv_out[b][:])

            # ---------- phase 4: attn^T = kv^T @ phi(q)^T per (b, pair) ----------
            for b in range(B):
                for h in range(H):
                    pr = h // 2
                    rows = slice((h % 2) * HD, (h % 2 + 1) * HD)
                    slot = pr * HD
                    bdc = pr * P + (h % 2) * HD
                    nc.vector.tensor_copy(out=kv_bd[b][rows, bdc:bdc + HD],
                                          in_=kv_rd[b][rows, slot:slot + HD])
                for pr in range(H // 2):
                    ap = psum.tile([P, NB], fp32, tag="pp")
                    bds = pr * P
                    nc.tensor.matmul(
                        ap[:],
                        lhsT=kv_bd[b][:, bds:bds + P],
                        rhs=phiq[:, pr, b * NB:(b + 1) * NB],
                        start=True, stop=True)
                    nc.scalar.activation(
                        out=attnt[:, pr, b * NB:(b + 1) * NB],
                        in_=ap[:], func=AF.Copy)

            # ---------- phase 5: out = attn @ Wo + bo ----------
            for g in range(SCH):
                for n in range(ND):
                    ops = psum.tile([P, 512], fp32, tag="pp")
                    for kc in range(KC):
                        nc.tensor.matmul(
                            ops[:], lhsT=attnt[:, kc, g * P:(g + 1) * P],
                            rhs=wo[n][:, kc, :],
                            start=(kc == 0),
                            stop=(not has_bias and kc == KC - 1))
                    if has_bias:
                        nc.tensor.matmul(ops[:], lhsT=ones[:],
                                         rhs=bo2[:, s512(n)],
                                         start=False, stop=True)
                    osb = outp.tile([P, 512], fp32, tag="osb")
                    nc.scalar.activation(out=osb[:], in_=ops[:], func=AF.Copy)
                    nc.sync.dma_start(out_d[g * P:(g + 1) * P, s512(n)], osb[:])

    nc.compile()
    return nc


def _get_nc(has_bias):
    key = ("nc", has_bias)
    if key not in _cache:
        _cache[key] = _build(has_bias)
    return _cache[key]


def _has_bias(inputs):
    return any(np.any(np.asarray(inputs[k], np.float32))
               for k in ("bk", "bv", "bo"))


def _make_in_maps(inputs, has_bias):
    bf16 = ml_dtypes.bfloat16
    x = np.asarray(inputs["x"], dtype=np.float32)
    ws = {k: np.ascontiguousarray(np.asarray(inputs[k], np.float32).astype(bf16))
          for k in ("Wq", "Wk", "Wv", "Wo")}
    bq = np.asarray(inputs["bq"], np.float32)
    bqc = np.ascontiguousarray(bq.reshape(KC, P).T.astype(np.float32))
    brow = {k: np.ascontiguousarray(
                np.asarray(inputs[k], np.float32).astype(bf16).reshape(1, DM))
            for k in ("bk", "bv", "bo")}
    xb = x.astype(bf16)
    in_maps = []
    for c in range(N_CORES):
        xs = np.ascontiguousarray(
            xb[:, c * SC:(c + 1) * SC, :].reshape(R, DM).T)
        m = {
            "x": xs,
            "wq": ws["Wq"], "wk": ws["Wk"], "wv": ws["Wv"], "wo": ws["Wo"],
            "bqc": bqc,
        }
        if has_bias:
            m.update({"bk2": brow["bk"], "bv2": brow["bv"],
                      "bo2": brow["bo"]})
        in_maps.append(m)
    return in_maps


def _run(inputs, **kw):
    from concourse import bass_utils
    hb = _has_bias(inputs)
    nc = _get_nc(hb)
    in_maps = _make_in_maps(inputs, hb)
    res = bass_utils.run_bass_kernel_spmd(
        nc, in_maps, core_ids=list(range(N_CORES)), **kw)
    out = np.empty((B, S, DM), np.float32)
    for c in range(N_CORES):
        out[:, c * SC:(c + 1) * SC, :] = res.results[c]["out"].reshape(B, SC, DM)
    return out, res


def kernel(**inputs) -> np.ndarray:
    out, _ = _run(inputs)
    return out

